# revision 10
# baseline (speedup 1.0000x reference)
"""Trainium2 Bass kernel for nn_Attention2 (attention-gated blend of Z_l/Z_g).

Reference math:
    Q     = Z_o @ W.T + b                      # [N, 512]
    att_l = Q @ colsum(Z_l)                    # [N]
    att_g = Q @ colsum(Z_g)                    # [N]
    att   = softmax([att_l, att_g], axis=1)    # [N, 2]
    out   = Z_l * att[:, 0:1] + Z_g * att[:, 1:2]

Only d = att_l - att_g matters (2-way softmax == sigmoid), and it folds:
    s = colsum(Z_l) - colsum(Z_g)              # [512]
    u = W.T @ s                                # [512]
    c = b . s                                  # scalar
    d = Z_o @ u + c                            # [N]
    out = Z_l * sigmoid(d) + Z_g * sigmoid(-d)

This removes the O(N*512*512) matmul; the kernel is HBM-bound.

Sharding: data-parallel over rows, N/8 rows per core. Two SPMD launches:
  A: per-core partial s = colsum(Z_l) - colsum(Z_g) via TensorE ones-matmul
     accumulation into one PSUM row -> [1, 512] per core.
  host: reduce partials (f64), u = W.T @ s, c = b . s  (tiny: 512x512 matvec)
  B: streams Z_o/Z_l/Z_g row tiles; d = rowwise_dot(Z_o, u) (+c via sigmoid
     bias); out = Z_l*sig(d) + Z_g*sig(-d); writes out.
"""

import numpy as np

import concourse.bacc as bacc
import concourse.mybir as mybir
import concourse.tile as tile
from concourse.bass_utils import run_bass_kernel_spmd

N_CORES = 8
N_TOTAL = 100000
CH = 512
SHARD = N_TOTAL // N_CORES  # 12500
P = 128


def build_nc_a(shard=SHARD, n_cores=N_CORES, bufs=6):
    f32 = mybir.dt.float32
    nc = bacc.Bacc(
        "TRN2",
        target_bir_lowering=False,
        debug=False,
        enable_asserts=False,
        num_devices=n_cores,
    )
    zl_d = nc.dram_tensor("Z_l", [shard, CH], f32, kind="ExternalInput")
    zg_d = nc.dram_tensor("Z_g", [shard, CH], f32, kind="ExternalInput")
    s_d = nc.dram_tensor("s_part", [1, CH], f32, kind="ExternalOutput")
    n_tiles = (shard + P - 1) // P
    with tile.TileContext(nc) as tc:
        with (
            tc.tile_pool(name="singles", bufs=1) as singles,
            tc.tile_pool(name="psum", bufs=1, space="PSUM") as psum,
            tc.tile_pool(name="p1", bufs=bufs) as p1,
        ):
            ones_col = singles.tile([P, 1], f32)
            neg_ones_col = singles.tile([P, 1], f32)
            nc.vector.memset(ones_col[:], 1.0)
            nc.vector.memset(neg_ones_col[:], -1.0)
            ps_s = psum.tile([1, CH], f32)
            for i in range(n_tiles):
                r0 = i * P
                R = min(P, shard - r0)
                zl = p1.tile([P, CH], f32, tag="zl")
                zg = p1.tile([P, CH], f32, tag="zg")
                nc.sync.dma_start(zl[:R], zl_d[r0 : r0 + R])
                nc.sync.dma_start(zg[:R], zg_d[r0 : r0 + R])
                nc.tensor.matmul(
                    ps_s[:], ones_col[:R], zl[:R], start=(i == 0), stop=False
                )
                nc.tensor.matmul(
                    ps_s[:],
                    neg_ones_col[:R],
                    zg[:R],
                    start=False,
                    stop=(i == n_tiles - 1),
                )
            s_sb = singles.tile([1, CH], f32)
            nc.vector.tensor_copy(s_sb[:], ps_s[:])
            nc.sync.dma_start(s_d[:, :], s_sb[:])
    nc.compile()
    return nc


def build_nc_b(shard=SHARD, n_cores=N_CORES, bufs=6):
    """Phase 2, core-ISA ops only; u_b/c_b/ncb arrive pre-broadcast from host."""
    f32 = mybir.dt.float32
    add = mybir.AluOpType.add
    AF = mybir.ActivationFunctionType
    nc = bacc.Bacc(
        "TRN2",
        target_bir_lowering=False,
        debug=False,
        enable_asserts=False,
        num_devices=n_cores,
    )
    zo_d = nc.dram_tensor("Z_o", [shard, CH], f32, kind="ExternalInput")
    zl_d = nc.dram_tensor("Z_l", [shard, CH], f32, kind="ExternalInput")
    zg_d = nc.dram_tensor("Z_g", [shard, CH], f32, kind="ExternalInput")
    ub_d = nc.dram_tensor("u_b", [P, CH], f32, kind="ExternalInput")
    cb_d = nc.dram_tensor("c_b", [P, 32], f32, kind="ExternalInput")
    ncb_d = nc.dram_tensor("ncb", [P, 32], f32, kind="ExternalInput")
    out_d = nc.dram_tensor("out", [shard, CH], f32, kind="ExternalOutput")
    n_tiles = (shard + P - 1) // P
    with tile.TileContext(nc) as tc:
        with (
            tc.tile_pool(name="singles", bufs=1) as singles,
            tc.tile_pool(name="p2", bufs=bufs) as p2,
            tc.tile_pool(name="small2", bufs=bufs) as small2,
        ):
            u_b = singles.tile([P, CH], f32)
            nc.sync.dma_start(u_b[:], ub_d[:, :])
            c_b = singles.tile([P, 32], f32)
            nc.sync.dma_start(c_b[:], cb_d[:, :])
            ncb = singles.tile([P, 32], f32)
            nc.sync.dma_start(ncb[:], ncb_d[:, :])
            for i in range(n_tiles):
                r0 = i * P
                R = min(P, shard - r0)
                zo = p2.tile([P, CH], f32, tag="zo")
                zl = p2.tile([P, CH], f32, tag="zl2")
                zg = p2.tile([P, CH], f32, tag="zg2")
                nc.sync.dma_start(zo[:R], zo_d[r0 : r0 + R])
                nc.sync.dma_start(zl[:R], zl_d[r0 : r0 + R])
                nc.sync.dma_start(zg[:R], zg_d[r0 : r0 + R])
                prod = p2.tile([P, CH], f32, tag="prod")
                nc.vector.tensor_mul(prod[:R], zo[:R], u_b[:R])
                d_t = small2.tile([P, 1], f32, tag="d")
                nc.vector.tensor_reduce(
                    d_t[:R], prod[:R], axis=mybir.AxisListType.X, op=add
                )
                frac = small2.tile([P, 1], f32, tag="fr")
                om = small2.tile([P, 1], f32, tag="om")
                nc.scalar.activation(
                    frac[:R], d_t[:R], AF.Sigmoid, bias=c_b[:R, 0:1], scale=1.0
                )
                nc.scalar.activation(
                    om[:R], d_t[:R], AF.Sigmoid, bias=ncb[:R, 0:1], scale=-1.0
                )
                t1 = p2.tile([P, CH], f32, tag="t1")
                nc.scalar.mul(t1[:R], zl[:R], frac[:R, 0:1])
                t2 = p2.tile([P, CH], f32, tag="t2")
                nc.scalar.mul(t2[:R], zg[:R], om[:R, 0:1])
                ot = p2.tile([P, CH], f32, tag="ot")
                nc.vector.tensor_add(ot[:R], t1[:R], t2[:R])
                nc.sync.dma_start(out_d[r0 : r0 + R], ot[:R])
    nc.compile()
    return nc


_CACHE = {}


def kernel(Z_o, Z_l, Z_g, W, b):
    Z_o = np.ascontiguousarray(np.asarray(Z_o, dtype=np.float32))
    Z_l = np.ascontiguousarray(np.asarray(Z_l, dtype=np.float32))
    Z_g = np.ascontiguousarray(np.asarray(Z_g, dtype=np.float32))
    W = np.ascontiguousarray(np.asarray(W, dtype=np.float32))
    b = np.ascontiguousarray(np.asarray(b, dtype=np.float32))
    if "a" not in _CACHE:
        _CACHE["a"] = build_nc_a()
        _CACHE["b"] = build_nc_b()
    nc_a, nc_b = _CACHE["a"], _CACHE["b"]
    sh = SHARD
    maps_a = [
        {"Z_l": Z_l[i * sh : (i + 1) * sh], "Z_g": Z_g[i * sh : (i + 1) * sh]}
        for i in range(N_CORES)
    ]
    res_a = run_bass_kernel_spmd(nc_a, maps_a, core_ids=list(range(N_CORES)))
    s = sum(r["s_part"][0].astype(np.float64) for r in res_a.results)
    u = (W.astype(np.float64).T @ s).astype(np.float32)
    c = np.float32(b.astype(np.float64) @ s)
    u_b = np.ascontiguousarray(np.broadcast_to(u, (P, CH)))
    c_b = np.full((P, 32), c, dtype=np.float32)
    ncb = -c_b
    maps_b = [
        {
            "Z_o": Z_o[i * sh : (i + 1) * sh],
            "Z_l": Z_l[i * sh : (i + 1) * sh],
            "Z_g": Z_g[i * sh : (i + 1) * sh],
            "u_b": u_b,
            "c_b": c_b,
            "ncb": ncb,
        }
        for i in range(N_CORES)
    ]
    res_b = run_bass_kernel_spmd(nc_b, maps_b, core_ids=list(range(N_CORES)))
    return np.concatenate([r["out"] for r in res_b.results], axis=0)
